# revision 3
# baseline (speedup 1.0000x reference)
"""Trainium2 Bass kernel for DWSpiralDeblock (gnn_message_passing).

Math (per batch sample b):
    edge   = x[:, trans_col, :] * trans_value            # [E, C]
    pooled = sum_j edge[row_map[:, j], :]                # [Vout, C]
    sp     = pooled[indices, :]                          # [Vout, S, C]
    dw     = einsum('vsc,cs->vc', sp, Wd) + bd
    out    = relu(dw @ Wp.T + bp)                        # [Vout, Cout]

Strategy: data-parallel across batch (8 samples -> 8 NeuronCores).
Stage 1 fuses the COO gather-scale with the 3-way row_map gather:
    pooled[u] = sum_j trans_value[row_map[u,j]] * x[trans_col[row_map[u,j]]]
using SWDGE dma_gathers from x (fp32 256B rows) + DVE multiply-accumulate
with host-precomputed composite indices/values. pooled is stored to DRAM as
bf16 padded to 128 channels (256B rows, the dma_gather granularity floor).
Stage 2 does dma_gathers from pooled per (s, v-chunk), DVE scale(Wd)+
accumulate in v-major layout, PE-transpose to channel-major, then a PE
matmul against [Wp.T; bp] (ones-row bias trick) with bd added via ACT bias
during the PSUM->SBUF copy; ACT relu and DMA out.

Each dma_gather is capped at GSZ indices: the SWDGE descriptor ring (the
DynamicDMAScratch SBUF carveout) overflows beyond ~half its capacity in
one instruction (HW-probed: 1024 ok / 2048 crash at the default 16 KiB).
"""

import sys

import numpy as np

if "/opt/trn_rl_repo" not in sys.path:
    sys.path.insert(0, "/opt/trn_rl_repo")

from contextlib import ExitStack

import concourse.tile as tile
from concourse import bacc, mybir
from concourse.bass_utils import run_bass_kernel_spmd
from concourse.masks import make_identity

B, VIN, C = 8, 4096, 64
VOUT, E, S, COUT = 16384, 49152, 9, 32

SCRATCH = 16384    # SWDGE descriptor-ring bytes per partition (HW ucode default)
GSZ = 1024         # indices per dma_gather instruction (HW-probed safe limit)
CU = 4096          # stage-1 u-chunk (DVE tile granularity)
NCH_U = VOUT // CU
JU = CU // 128
CV = 8192          # stage-2 v-chunk
NCH_V = VOUT // CV
JV = CV // 128

F32 = mybir.dt.float32
BF16 = mybir.dt.bfloat16
I16 = mybir.dt.int16

_PROGRAM = None


def _gathers(nc, out_tile, in_ap, idx_sb, block_idx, total):
    """Issue total/GSZ sub-gathers of GSZ indices into slices of out_tile.

    block_idx is the flat block number (each block = `total` indices) into
    the packed index tensor; sub-gather g covers rows [g*GSZ, (g+1)*GSZ) of
    the block and writes out_tile[:, g*(GSZ//128):(g+1)*(GSZ//128), :].
    """
    nsub = total // GSZ
    cols = GSZ // 16
    jg = GSZ // 128
    elem = out_tile.shape[-1]
    for g in range(nsub):
        a = (block_idx * nsub + g) * cols
        nc.gpsimd.dma_gather(
            out_ap=out_tile[:, g * jg:(g + 1) * jg, :],
            in_ap=in_ap,
            idxs_ap=idx_sb[:, a:a + cols],
            num_idxs=GSZ,
            num_idxs_reg=GSZ,
            elem_size=elem,
        )


def _build_program():
    nc = bacc.Bacc("TRN2", target_bir_lowering=False, debug=False,
                   num_devices=B, dynamic_dma_scratch_size=SCRATCH)

    xb = nc.dram_tensor("xb", [VIN, C], F32, kind="ExternalInput")
    idxcc = nc.dram_tensor("idxcc", [128, 3 * VOUT // 16], I16, kind="ExternalInput")
    cvw = nc.dram_tensor("cvw", [128, 3 * VOUT // 128], F32, kind="ExternalInput")
    idxsp = nc.dram_tensor("idxsp", [128, S * VOUT // 16], I16, kind="ExternalInput")
    wdrep = nc.dram_tensor("wdrep", [128, S * C], F32, kind="ExternalInput")
    wptbp = nc.dram_tensor("wptbp", [C + 1, COUT], F32, kind="ExternalInput")
    bdt = nc.dram_tensor("bdt", [C, 1], F32, kind="ExternalInput")
    out = nc.dram_tensor("out", [VOUT, COUT], F32, kind="ExternalOutput")

    with tile.TileContext(nc) as tc, ExitStack() as ctx:
        const = ctx.enter_context(tc.tile_pool(name="const", bufs=1))
        dram = ctx.enter_context(tc.tile_pool(name="dram", bufs=1, space="DRAM"))

        pooled = dram.tile([VOUT, 2 * C], BF16)

        idxcc_sb = const.tile([128, 3 * VOUT // 16], I16)
        nc.sync.dma_start(idxcc_sb[:], idxcc.ap()[:])
        idxsp_sb = const.tile([128, S * VOUT // 16], I16)
        nc.sync.dma_start(idxsp_sb[:], idxsp.ap()[:])
        cvw_sb = const.tile([128, 3 * VOUT // 128], F32)
        nc.sync.dma_start(cvw_sb[:], cvw.ap()[:])
        wd_sb = const.tile([128, S * C], F32)
        nc.sync.dma_start(wd_sb[:], wdrep.ap()[:])
        wpt_sb = const.tile([C + 1, COUT], F32)
        nc.sync.dma_start(wpt_sb[:], wptbp.ap()[:])
        bd_sb = const.tile([C, 1], F32)
        nc.sync.dma_start(bd_sb[:], bdt.ap()[:])
        ident = const.tile([128, 128], F32)
        make_identity(nc, ident[:])

        # ---- Stage 1: pooled[u] = sum_j cv[u,j] * x[cc[u,j], :] ----
        pooled_v = pooled[:].rearrange("(ch j p) d -> ch p j d", p=128, j=JU)
        with tc.tile_pool(name="s1", bufs=2) as s1, \
             tc.tile_pool(name="s1o", bufs=2) as s1o:
            for ch in range(NCH_U):
                xg = []
                for j in range(3):
                    g = s1.tile([128, JU, C], F32, tag=f"xg{j}")
                    _gathers(nc, g, xb.ap()[:], idxcc_sb, j * NCH_U + ch, CU)
                    xg.append(g)

                def cvb(j):
                    a = (j * NCH_U + ch) * JU
                    return cvw_sb[:, a:a + JU, None].to_broadcast([128, JU, C])

                acc = s1.tile([128, JU, C], F32, tag="acc")
                nc.vector.tensor_mul(acc[:], xg[0][:], cvb(0))
                m1 = s1.tile([128, JU, C], F32, tag="m")
                nc.vector.tensor_mul(m1[:], xg[1][:], cvb(1))
                nc.vector.tensor_add(acc[:], acc[:], m1[:])
                m2 = s1.tile([128, JU, C], F32, tag="m")
                nc.vector.tensor_mul(m2[:], xg[2][:], cvb(2))
                st = s1o.tile([128, JU, 2 * C], BF16, tag="st")
                nc.vector.memset(st[:, :, C:2 * C], 0.0)
                nc.vector.tensor_add(st[:, :, 0:C], acc[:], m2[:])
                nc.sync.dma_start(pooled_v[ch], st[:])

        # ---- Stage 2: spiral gather + depthwise + pointwise ----
        out_v = out.ap()[:].rearrange(
            "(ch g q p) o -> ch g p q o", p=128, q=4, g=JV // 4
        )
        with tc.tile_pool(name="sp", bufs=3) as spp, \
             tc.tile_pool(name="s2m", bufs=2) as s2m, \
             tc.tile_pool(name="s2acc", bufs=2) as s2acc, \
             tc.tile_pool(name="dw", bufs=2) as dwp, \
             tc.tile_pool(name="osb", bufs=2) as outp, \
             tc.tile_pool(name="psT", bufs=2, space="PSUM") as psTp, \
             tc.tile_pool(name="ps2", bufs=2, space="PSUM") as ps2p:
            for ch in range(NCH_V):
                acc = s2acc.tile([128, JV, C], F32, tag="acc")
                for s in range(S):
                    sp = spp.tile([128, JV, 2 * C], BF16, tag="sp")
                    _gathers(nc, sp, pooled[:], idxsp_sb, s * NCH_V + ch, CV)
                    wdb = wd_sb[:, None, s * C:(s + 1) * C].to_broadcast([128, JV, C])
                    if s == 0:
                        nc.vector.tensor_mul(acc[:], sp[:, :, 0:C], wdb)
                    else:
                        m = s2m.tile([128, JV, C], F32, tag="m")
                        nc.vector.tensor_mul(m[:], sp[:, :, 0:C], wdb)
                        nc.vector.tensor_add(acc[:], acc[:], m[:])
                for g in range(JV // 4):
                    psT = psTp.tile([C, 512], F32)
                    for q in range(4):
                        nc.tensor.transpose(
                            psT[:, q * 128:(q + 1) * 128],
                            acc[:, g * 4 + q, :],
                            ident[:],
                        )
                    dwT = dwp.tile([C + 1, 512], F32, tag="dwT")
                    nc.vector.memset(dwT[C:C + 1, :], 1.0)
                    nc.scalar.activation(
                        dwT[0:C, :], psT[:],
                        mybir.ActivationFunctionType.Identity,
                        bias=bd_sb[:],
                    )
                    ps2 = ps2p.tile([128, 4, COUT], F32)
                    for q in range(4):
                        nc.tensor.matmul(
                            ps2[:, q, :],
                            lhsT=dwT[:, q * 128:(q + 1) * 128],
                            rhs=wpt_sb[:],
                            start=True,
                            stop=True,
                        )
                    osb = outp.tile([128, 4, COUT], F32, tag="osb")
                    nc.scalar.activation(
                        osb[:], ps2[:], mybir.ActivationFunctionType.Relu
                    )
                    nc.sync.dma_start(out_v[ch, g], osb[:])

    nc.compile()
    return nc


def _wrap16(a):
    """One gather block's index layout: logical position i -> [i % 16,
    i // 16], the 16-partition block replicated 8x across 128 partitions."""
    return np.tile(np.ascontiguousarray(a.reshape(-1, 16).T), (8, 1))


def _wrap_blocks(a):
    """Split a into GSZ-sized sub-gather blocks, wrap each, concat."""
    return np.concatenate(
        [_wrap16(a[g * GSZ:(g + 1) * GSZ]) for g in range(len(a) // GSZ)],
        axis=1,
    )


def _prep_shared(trans_col, trans_value, row_map, indices, Wd, bd, Wp, bp):
    tcol = np.asarray(trans_col)
    tval = np.asarray(trans_value, dtype=np.float32)
    rm = np.asarray(row_map)
    idx = np.asarray(indices)

    cc = tcol[rm].astype(np.int16)          # [VOUT, 3]
    cv = tval[rm].astype(np.float32)        # [VOUT, 3]

    idxcc = np.concatenate(
        [_wrap_blocks(cc[ch * CU:(ch + 1) * CU, j])
         for j in range(3) for ch in range(NCH_U)],
        axis=1,
    )
    cvw = np.concatenate(
        [np.ascontiguousarray(cv[ch * CU:(ch + 1) * CU, j].reshape(JU, 128).T)
         for j in range(3) for ch in range(NCH_U)],
        axis=1,
    ).astype(np.float32)
    idxsp = np.concatenate(
        [_wrap_blocks(idx[ch * CV:(ch + 1) * CV, s].astype(np.int16))
         for s in range(S) for ch in range(NCH_V)],
        axis=1,
    )
    wd = np.asarray(Wd, dtype=np.float32)        # [C, S]
    wdrep = np.tile(wd.T.reshape(1, S * C), (128, 1)).astype(np.float32)
    wptbp = np.concatenate(
        [np.asarray(Wp, dtype=np.float32).T,
         np.asarray(bp, dtype=np.float32)[None, :]],
        axis=0,
    ).astype(np.float32)
    bdt = np.asarray(bd, dtype=np.float32).reshape(C, 1)
    return dict(idxcc=idxcc, cvw=cvw, idxsp=idxsp, wdrep=wdrep,
                wptbp=wptbp, bdt=bdt)


def kernel(x, trans_row, trans_col, trans_value, row_map, indices,
           Wd, bd, Wp, bp):
    global _PROGRAM
    if _PROGRAM is None:
        _PROGRAM = _build_program()
    nc = _PROGRAM

    shared = _prep_shared(trans_col, trans_value, row_map, indices,
                          Wd, bd, Wp, bp)
    x = np.asarray(x, dtype=np.float32)
    in_maps = [
        {"xb": np.ascontiguousarray(x[b]), **shared}
        for b in range(B)
    ]
    res = run_bass_kernel_spmd(nc, in_maps, list(range(B)))
    return np.stack([res.results[b]["out"] for b in range(B)], axis=0)


if __name__ == "__main__":
    _build_program()
    print("build ok")


# revision 5
# speedup vs baseline: 4.6776x; 4.6776x over previous
"""Trainium2 Bass kernel for DWSpiralDeblock (gnn_message_passing).

Math (per batch sample b):
    edge   = x[:, trans_col, :] * trans_value            # [E, C]
    pooled = sum_j edge[row_map[:, j], :]                # [Vout, C]
    sp     = pooled[indices, :]                          # [Vout, S, C]
    dw     = einsum('vsc,cs->vc', sp, Wd) + bd
    out    = relu(dw @ Wp.T + bp)                        # [Vout, Cout]

Strategy: data-parallel across batch (8 samples -> 8 NeuronCores).
Stage 1 fuses the COO gather-scale with the 3-way row_map gather:
    pooled[u] = sum_j trans_value[row_map[u,j]] * x[trans_col[row_map[u,j]]]
using SWDGE dma_gathers from x (fp32 256B rows) + DVE multiply-accumulate
with host-precomputed composite indices/values. pooled is stored to DRAM as
bf16 padded to 128 channels (256B rows, the dma_gather granularity floor).
Stage 2 does dma_gathers from pooled per (s, v-chunk), DVE scale(Wd)+
accumulate in v-major layout, PE-transpose to channel-major, then a PE
matmul against [Wp.T; bp] (ones-row bias trick) with bd added via ACT bias
during the PSUM->SBUF copy; ACT relu and DMA out.

Each dma_gather is capped at GSZ indices: the SWDGE descriptor ring (the
DynamicDMAScratch SBUF carveout) overflows beyond ~half its capacity in
one instruction (HW-probed: 1024 ok / 2048 crash at the default 16 KiB).
"""

import sys

import numpy as np

if "/opt/trn_rl_repo" not in sys.path:
    sys.path.insert(0, "/opt/trn_rl_repo")

from contextlib import ExitStack

import concourse.tile as tile
from concourse import bacc, mybir
from concourse.bass_utils import run_bass_kernel_spmd
from concourse.masks import make_identity

B, VIN, C = 8, 4096, 64
VOUT, E, S, COUT = 16384, 49152, 9, 32

SCRATCH = 16384    # SWDGE descriptor-ring bytes per partition (HW ucode default)
GSZ = 1024         # indices per dma_gather instruction (HW-probed safe limit)
NQ = 4             # SWDGE queues (desc-gen parallelism, HW-probed ~3.7x)
CU = 4096          # stage-1 u-chunk (DVE tile granularity)
NCH_U = VOUT // CU
JU = CU // 128
CV = 8192          # stage-2 v-chunk
NCH_V = VOUT // CV
JV = CV // 128

F32 = mybir.dt.float32
BF16 = mybir.dt.bfloat16
I16 = mybir.dt.int16

_PROGRAM = None


_QCTR = [0]


def _gathers(nc, out_tile, in_ap, idx_sb, block_idx, total):
    """Issue total/GSZ sub-gathers of GSZ indices into slices of out_tile.

    block_idx is the flat block number (each block = `total` indices) into
    the packed index tensor; sub-gather g covers rows [g*GSZ, (g+1)*GSZ) of
    the block and writes out_tile[:, g*(GSZ//128):(g+1)*(GSZ//128), :].
    """
    nsub = total // GSZ
    cols = GSZ // 16
    jg = GSZ // 128
    elem = out_tile.shape[-1]
    for g in range(nsub):
        a = (block_idx * nsub + g) * cols
        nc.gpsimd.dma_gather(
            out_ap=out_tile[:, g * jg:(g + 1) * jg, :],
            in_ap=in_ap,
            idxs_ap=idx_sb[:, a:a + cols],
            num_idxs=GSZ,
            num_idxs_reg=GSZ,
            elem_size=elem,
            queue_num=_QCTR[0] % NQ,
        )
        _QCTR[0] += 1


def _build_program():
    _QCTR[0] = 0
    nc = bacc.Bacc("TRN2", target_bir_lowering=False, debug=False,
                   num_devices=B, dynamic_dma_scratch_size=SCRATCH,
                   num_swdge_queues=NQ)

    xb = nc.dram_tensor("xb", [VIN, C], F32, kind="ExternalInput")
    idxcc = nc.dram_tensor("idxcc", [128, 3 * VOUT // 16], I16, kind="ExternalInput")
    cvw = nc.dram_tensor("cvw", [128, 3 * VOUT // 128], F32, kind="ExternalInput")
    idxsp = nc.dram_tensor("idxsp", [128, S * VOUT // 16], I16, kind="ExternalInput")
    wdrep = nc.dram_tensor("wdrep", [128, S * C], F32, kind="ExternalInput")
    wptbp = nc.dram_tensor("wptbp", [C + 1, COUT], F32, kind="ExternalInput")
    bdt = nc.dram_tensor("bdt", [C, 1], F32, kind="ExternalInput")
    out = nc.dram_tensor("out", [VOUT, COUT], F32, kind="ExternalOutput")

    with tile.TileContext(nc) as tc, ExitStack() as ctx:
        const = ctx.enter_context(tc.tile_pool(name="const", bufs=1))
        dram = ctx.enter_context(tc.tile_pool(name="dram", bufs=1, space="DRAM"))

        pooled = dram.tile([VOUT, 2 * C], BF16)

        idxcc_sb = const.tile([128, 3 * VOUT // 16], I16)
        nc.sync.dma_start(idxcc_sb[:], idxcc.ap()[:])
        idxsp_sb = const.tile([128, S * VOUT // 16], I16)
        nc.sync.dma_start(idxsp_sb[:], idxsp.ap()[:])
        cvw_sb = const.tile([128, 3 * VOUT // 128], F32)
        nc.sync.dma_start(cvw_sb[:], cvw.ap()[:])
        wd_sb = const.tile([128, S * C], F32)
        nc.sync.dma_start(wd_sb[:], wdrep.ap()[:])
        wpt_sb = const.tile([C + 1, COUT], F32)
        nc.sync.dma_start(wpt_sb[:], wptbp.ap()[:])
        bd_sb = const.tile([C, 1], F32)
        nc.sync.dma_start(bd_sb[:], bdt.ap()[:])
        ident = const.tile([128, 128], F32)
        make_identity(nc, ident[:])

        # ---- Stage 1: pooled[u] = sum_j cv[u,j] * x[cc[u,j], :] ----
        pooled_v = pooled[:].rearrange("(ch j p) d -> ch p j d", p=128, j=JU)
        with tc.tile_pool(name="s1", bufs=2) as s1, \
             tc.tile_pool(name="s1o", bufs=2) as s1o:
            for ch in range(NCH_U):
                xg = []
                for j in range(3):
                    g = s1.tile([128, JU, C], F32, tag=f"xg{j}")
                    _gathers(nc, g, xb.ap()[:], idxcc_sb, j * NCH_U + ch, CU)
                    xg.append(g)

                def cvb(j):
                    a = (j * NCH_U + ch) * JU
                    return cvw_sb[:, a:a + JU, None].to_broadcast([128, JU, C])

                acc = s1.tile([128, JU, C], F32, tag="acc")
                nc.vector.tensor_mul(acc[:], xg[0][:], cvb(0))
                m1 = s1.tile([128, JU, C], F32, tag="m")
                nc.vector.tensor_mul(m1[:], xg[1][:], cvb(1))
                nc.vector.tensor_add(acc[:], acc[:], m1[:])
                m2 = s1.tile([128, JU, C], F32, tag="m")
                nc.vector.tensor_mul(m2[:], xg[2][:], cvb(2))
                st = s1o.tile([128, JU, 2 * C], BF16, tag="st")
                nc.vector.memset(st[:, :, C:2 * C], 0.0)
                nc.vector.tensor_add(st[:, :, 0:C], acc[:], m2[:])
                nc.sync.dma_start(pooled_v[ch], st[:])

        # ---- Stage 2: spiral gather + depthwise + pointwise ----
        out_v = out.ap()[:].rearrange(
            "(ch g q p) o -> ch g p q o", p=128, q=4, g=JV // 4
        )
        with tc.tile_pool(name="sp", bufs=3) as spp, \
             tc.tile_pool(name="s2m", bufs=2) as s2m, \
             tc.tile_pool(name="s2acc", bufs=2) as s2acc, \
             tc.tile_pool(name="dw", bufs=2) as dwp, \
             tc.tile_pool(name="osb", bufs=2) as outp, \
             tc.tile_pool(name="psT", bufs=2, space="PSUM") as psTp, \
             tc.tile_pool(name="ps2", bufs=2, space="PSUM") as ps2p:
            for ch in range(NCH_V):
                acc = s2acc.tile([128, JV, C], F32, tag="acc")
                for s in range(S):
                    sp = spp.tile([128, JV, 2 * C], BF16, tag="sp")
                    _gathers(nc, sp, pooled[:], idxsp_sb, s * NCH_V + ch, CV)
                    wdb = wd_sb[:, None, s * C:(s + 1) * C].to_broadcast([128, JV, C])
                    if s == 0:
                        nc.vector.tensor_mul(acc[:], sp[:, :, 0:C], wdb)
                    else:
                        m = s2m.tile([128, JV, C], F32, tag="m")
                        nc.vector.tensor_mul(m[:], sp[:, :, 0:C], wdb)
                        nc.vector.tensor_add(acc[:], acc[:], m[:])
                for g in range(JV // 4):
                    psT = psTp.tile([C, 512], F32)
                    for q in range(4):
                        nc.tensor.transpose(
                            psT[:, q * 128:(q + 1) * 128],
                            acc[:, g * 4 + q, :],
                            ident[:],
                        )
                    dwT = dwp.tile([C + 1, 512], F32, tag="dwT")
                    nc.vector.memset(dwT[C:C + 1, :], 1.0)
                    nc.scalar.activation(
                        dwT[0:C, :], psT[:],
                        mybir.ActivationFunctionType.Identity,
                        bias=bd_sb[:],
                    )
                    ps2 = ps2p.tile([128, 4, COUT], F32)
                    for q in range(4):
                        nc.tensor.matmul(
                            ps2[:, q, :],
                            lhsT=dwT[:, q * 128:(q + 1) * 128],
                            rhs=wpt_sb[:],
                            start=True,
                            stop=True,
                        )
                    osb = outp.tile([128, 4, COUT], F32, tag="osb")
                    nc.scalar.activation(
                        osb[:], ps2[:], mybir.ActivationFunctionType.Relu
                    )
                    nc.sync.dma_start(out_v[ch, g], osb[:])

    nc.compile()
    return nc


def _wrap16(a):
    """One gather block's index layout: logical position i -> [i % 16,
    i // 16], the 16-partition block replicated 8x across 128 partitions."""
    return np.tile(np.ascontiguousarray(a.reshape(-1, 16).T), (8, 1))


def _wrap_blocks(a):
    """Split a into GSZ-sized sub-gather blocks, wrap each, concat."""
    return np.concatenate(
        [_wrap16(a[g * GSZ:(g + 1) * GSZ]) for g in range(len(a) // GSZ)],
        axis=1,
    )


def _prep_shared(trans_col, trans_value, row_map, indices, Wd, bd, Wp, bp):
    tcol = np.asarray(trans_col)
    tval = np.asarray(trans_value, dtype=np.float32)
    rm = np.asarray(row_map)
    idx = np.asarray(indices)

    cc = tcol[rm].astype(np.int16)          # [VOUT, 3]
    cv = tval[rm].astype(np.float32)        # [VOUT, 3]

    idxcc = np.concatenate(
        [_wrap_blocks(cc[ch * CU:(ch + 1) * CU, j])
         for j in range(3) for ch in range(NCH_U)],
        axis=1,
    )
    cvw = np.concatenate(
        [np.ascontiguousarray(cv[ch * CU:(ch + 1) * CU, j].reshape(JU, 128).T)
         for j in range(3) for ch in range(NCH_U)],
        axis=1,
    ).astype(np.float32)
    idxsp = np.concatenate(
        [_wrap_blocks(idx[ch * CV:(ch + 1) * CV, s].astype(np.int16))
         for s in range(S) for ch in range(NCH_V)],
        axis=1,
    )
    wd = np.asarray(Wd, dtype=np.float32)        # [C, S]
    wdrep = np.tile(wd.T.reshape(1, S * C), (128, 1)).astype(np.float32)
    wptbp = np.concatenate(
        [np.asarray(Wp, dtype=np.float32).T,
         np.asarray(bp, dtype=np.float32)[None, :]],
        axis=0,
    ).astype(np.float32)
    bdt = np.asarray(bd, dtype=np.float32).reshape(C, 1)
    return dict(idxcc=idxcc, cvw=cvw, idxsp=idxsp, wdrep=wdrep,
                wptbp=wptbp, bdt=bdt)


def kernel(x, trans_row, trans_col, trans_value, row_map, indices,
           Wd, bd, Wp, bp):
    global _PROGRAM
    if _PROGRAM is None:
        _PROGRAM = _build_program()
    nc = _PROGRAM

    shared = _prep_shared(trans_col, trans_value, row_map, indices,
                          Wd, bd, Wp, bp)
    x = np.asarray(x, dtype=np.float32)
    in_maps = [
        {"xb": np.ascontiguousarray(x[b]), **shared}
        for b in range(B)
    ]
    res = run_bass_kernel_spmd(nc, in_maps, list(range(B)))
    return np.stack([res.results[b]["out"] for b in range(B)], axis=0)


if __name__ == "__main__":
    _build_program()
    print("build ok")
